# revision 45
# baseline (speedup 1.0000x reference)
"""Masked dot-product attention on 8 Trainium2 NeuronCores (Bass/Tile).

Problem: queries/keys/values [32, 1024, 128] f32, valid_lens [32] i32.
  out = softmax(mask(Q K^T / sqrt(128))) V        (key-padding prefix mask)

Strategy (fragment-parallel flash decomposition, one SPMD program):
  * The unit of work is a key CHUNK (128 keys) of one batch. Since the
    kernel's softmax uses no running-max (scores ~ N(0,1), exp is safe),
    a batch's chunks can be split across cores: each fragment produces a
    partial out^T = V_r^T @ exp(S_r^T) and partial denominator sums; the
    host adds partials across fragments and divides. This drops per-core
    work from the sum of per-slot whole-batch maxima (20 chunks) to the
    exact floor ceil(total_chunks/8) (17 for the reference lens).
  * plan() finds a slot profile (p_1..p_S) and an exact cutting of the
    32 batches into fragments filling all 8*S slot instances (DP when
    zero slack, greedy otherwise, whole-batch fallback).
  * Per (core, slot): one fused input bundle [qt | kt | vp] (bf16, host
    pre-transposed) loaded with ONE dma_start on the Sync HWDGE ring in
    ascending-slot order; slot 0's bundle also carries the exp bias table
    mb (0 / -1e6, bf16 is exact enough: exp(x-999424)==0) and the
    sums-matmul constant columns. This cuts ~14 input issues (~0.6us of
    queue time each) to ~6 and keeps the ACT queue free of DMA issues so
    the first real exp can run right after ACT_TABLE_LOAD + dummy exp.
  * Scores are computed transposed: S^T[k, q] = kt_chunk.T @ qt with k on
    partitions; the prefix mask folds into the exp bias for free; probs
    are bf16; out^T accumulates over the fragment's chunks in two
    half-PSUM accumulators that free independently; the denominator uses
    a DVE running-sum chain + one deferred 2-matmul partition-reduction
    per fragment into a [2, 512] PSUM bank (see baseline notes below).
  * PE + ACT warmup: dummy matmuls on a memset tile (no DMA dependency)
    bridge the initial DMA window so the HAM clock-gate reaches 8/8, and
    a dummy exp pulls the one-time ACT_TABLE_LOAD forward.
  * The chunk loop is software-pipelined with 2-deep score lookahead so
    ACT never starves; epilogue PSUM->SBUF copies run on DVE (Scalar for
    the final slot, whose exps are done) before the fragment-final add.

Host gather: out[b] = (sum_frag out_frag / sum_frag sums_frag)^T, f32.
"""

import math

import ml_dtypes
import numpy as np

import concourse.bacc as bacc
import concourse.bass as bass
import concourse.mybir as mybir
import concourse.tile as tile
from concourse.bass_utils import run_bass_kernel_spmd

B, Q, K, D = 32, 1024, 1024, 128
N_CORES = 8
PART = 128          # partition size / key chunk size
NCHUNK = K // PART
MASK_BIAS = -1.0e6
INV_SQRT_D = 1.0 / math.sqrt(D)
F32 = mybir.dt.float32
BF16 = mybir.dt.bfloat16
NP_BF16 = ml_dtypes.bfloat16
N_WARM_MM = 7       # dummy PE matmuls (512 cols each): bridge the initial
                    # DMA window (~entry+4.4us until the first input piece
                    # lands) so the HAM activity stays unbroken
P_BUFS = 12         # probs-tile ring size

_NC_CACHE: dict = {}


def _w_cols(profile, s):
    # w piece: slot 0 [kt | qt | mb | cst]; others [kt | qt_h0]
    S = len(profile)
    if s == 0:
        return profile[s] * PART + Q + S * NCHUNK + 4
    return profile[s] * PART + 512


def _kv_cols(profile, s):
    # kv piece: [vp_all] (slots >= 4 also carry their qt_h1 up front)
    return profile[s] * PART + (512 if s >= 4 else 0)


def build_nc(profile: tuple) -> bass.Bass:
    """Build the SPMD Bass program for a slot chunk-count profile."""
    S = len(profile)
    nc = bacc.Bacc()
    # Three DMA rails. All completions share a global ~11.2us wall (the
    # first DMA completion lands ~4.5us after entry no matter how small),
    # then each rail streams ~110-140 GB/s. Everything the exp stream
    # blocks on rides Sync in consumption order; the ACT HWDGE ring is
    # idle between the dummy exp and the first real exp, so it carries
    # the early slots' qt_h1; SWDGE carries every vp (needed ~1us later
    # than its slot's scores) plus the late slots' qt_h1:
    #   w{s}  (Sync)   = [kt_all | qt_h0]  (slot 0: full qt + [mb | cst])
    #   qh{s} (Scalar) = [qt_h1]           (slots 1..3)
    #   kv{s} (SWDGE)  = [vp_all]          (slots 4+: [qt_h1 | vp_all])
    ws, kvs, qhs = [], [], {}
    for s, p in enumerate(profile):
        ws.append(nc.declare_dram_parameter(
            f"w{s}", [PART, _w_cols(profile, s)], BF16, isOutput=False))
        kvs.append(nc.declare_dram_parameter(
            f"kv{s}", [PART, _kv_cols(profile, s)], BF16, isOutput=False))
        if 1 <= s <= 3:
            qhs[s] = nc.declare_dram_parameter(
                f"qh{s}", [PART, 512], BF16, isOutput=False)
    out = nc.declare_dram_parameter("out", [S, PART, Q], BF16, isOutput=True)
    sums_out = nc.declare_dram_parameter("sums", [S, 2, 512], F32, isOutput=True)
    # Non-final slots ship their raw running prob-sum tile [128(k), 1024(q)]
    # instead of PE-reducing it on-device; the host does the 128-partition
    # reduction (identical numerics: f32 accumulation of bf16 terms). This
    # removes the per-slot sums matmuls from the co-critical PE stream and
    # the per-slot copies from DVE.
    csum = nc.declare_dram_parameter("csum", [max(S - 1, 1), PART, Q], BF16,
                                     isOutput=True)

    with tile.TileContext(nc) as tc:
        with (
            tc.tile_pool(name="sb", bufs=1) as sb,
            tc.tile_pool(name="ps", bufs=1, space="PSUM") as ps,
        ):
            # Warmup with no DMA dependency: memset a tile, then dummy
            # matmuls (HAM warm) + a dummy exp (one-time exp table load)
            # while the first inputs stream in.
            warm_sb = sb.tile([PART, 512], BF16, tag="warm")
            nc.vector.memset(warm_sb, 1.0)
            warm_ps = ps.tile([PART, 512], F32, tag="fill", bufs=1)
            for _ in range(N_WARM_MM):
                nc.tensor.matmul(
                    warm_ps, warm_sb[:, 0:PART], warm_sb, start=True, stop=True
                )
            warm_act = sb.tile([PART, 1], F32, tag="warm_act")
            nc.scalar.activation(
                warm_act,
                warm_sb[:, 0:1],
                mybir.ActivationFunctionType.Exp,
                scale=0.0,
            )
            # Early slots' qt_h1 on the ACT HWDGE ring: issued in the
            # window between the dummy exp and the first real exp.
            qh_sb = {}
            for s in sorted(qhs):
                qh_sb[s] = sb.tile([PART, 512], BF16, tag=f"qh{s}",
                                   name=f"qh{s}")
                nc.scalar.dma_start(out=qh_sb[s], in_=qhs[s][:, :])

            # Input streaming, ascending slot order (slot 0 is smallest).
            # Both rails pipeline their transfers FIFO, so pieces are
            # ordered by first-use time; all issued up front.
            w_sb, kv_sb = [], []
            for s, p in enumerate(profile):
                w_sb.append(sb.tile([PART, _w_cols(profile, s)], BF16,
                                    tag=f"w{s}", name=f"w{s}"))
                kv_sb.append(sb.tile([PART, _kv_cols(profile, s)], BF16,
                                     tag=f"kv{s}", name=f"kv{s}"))
            for s in range(S):
                nc.sync.dma_start(out=w_sb[s], in_=ws[s][:, :])
            for s in range(S):
                nc.gpsimd.dma_start(out=kv_sb[s], in_=kvs[s][:, :])
            mb_off = profile[0] * PART + Q
            mb_sb = w_sb[0][:, mb_off:mb_off + S * NCHUNK]
            cst_sb = w_sb[0][:, mb_off + S * NCHUNK:mb_off + S * NCHUNK + 4]

            def qt_w(s, h):
                if s == 0 or h == 0:
                    off = profile[s] * PART + h * 512
                    return w_sb[s][:, off:off + 512]
                if s in qh_sb:
                    return qh_sb[s][:, 0:512]
                return kv_sb[s][:, 0:512]

            def kt_w(s, c):
                return w_sb[s][:, c * PART:(c + 1) * PART]

            def vp_w(s, c):
                off = 512 if s >= 4 else 0
                return kv_sb[s][:, off + c * PART:off + (c + 1) * PART]

            # Flat chunk stream across slots with 2-deep score lookahead:
            # the in-order PE queue must see the next chunks' score
            # matmuls BEFORE a slot-boundary AV matmul that may stall on
            # the PSUM accumulator release.
            stream = [(s, c) for s in range(S) for c in range(profile[s])]

            def s_alloc(s, c):
                return ps.tile([PART, Q], F32, tag="s", bufs=2,
                               name=f"s_s{s}c{c}")

            def s_mm_h(s, c, s_ps, h):
                # h1 reads qt_h1 from the kv piece (slower rail), so it is
                # emitted at shallower lookahead than h0 to keep the
                # in-order PE queue from head-of-line blocking on it.
                nc.tensor.matmul(
                    s_ps[:, h * 512:(h + 1) * 512],
                    kt_w(s, c),
                    qt_w(s, h),
                    start=True,
                    stop=True,
                )

            def p_tile(nm):
                return sb.tile([PART, Q], BF16, tag="p", bufs=P_BUFS, name=nm)

            def sums_mms(sums_ps, rhs_t, st, sp):
                # Rows [sum of h0 cols; sum of h1 cols] into one PSUM
                # bank: lhsT columns are [1,0] and [0,1] of cst.
                nc.tensor.matmul(
                    sums_ps[0:2, 0:512],
                    cst_sb[:, 0:2],
                    rhs_t[:, 0:512],
                    start=st,
                    stop=False,
                )
                nc.tensor.matmul(
                    sums_ps[0:2, 0:512],
                    cst_sb[:, 2:4],
                    rhs_t[:, 512:1024],
                    start=False,
                    stop=sp,
                )

            def sums_epilogue(s, sums_ps):
                sums_sb = sb.tile(
                    [2, 512], F32, tag="sums_sb", bufs=2, name=f"sums_sb{s}"
                )
                # Final slot: ACT is idle after the last exp; copy on
                # Scalar, issue on Sync (the Scalar-ring last-DMA issue
                # slice measures ~1.35us vs ~0.6 on Sync).
                nc.scalar.copy(sums_sb, sums_ps)
                nc.sync.dma_start(out=sums_out[s], in_=sums_sb)

            def s_mms(s, c):
                t = s_alloc(s, c)
                s_mm_h(s, c, t, 0)
                s_mm_h(s, c, t, 1)
                return t

            s_tiles = {}
            for j in range(min(2, len(stream))):
                s_tiles[stream[j]] = s_mms(*stream[j])
            acc = {}
            run = {}  # per-slot running prob-sum tile
            for i, (s, c) in enumerate(stream):
                cap = profile[s]
                if c == 0:
                    # Two independent half-accumulators (one PSUM bank
                    # each): each half frees as soon as its own epilogue
                    # copy is done.
                    out_ps = (
                        ps.tile([PART, 512], F32, tag="outA", bufs=1,
                                name=f"outA_s{s}"),
                        ps.tile([PART, 512], F32, tag="outB", bufs=1,
                                name=f"outB_s{s}"),
                    )
                    sums_ps = ps.tile(
                        [2, 512], F32, tag="sums", bufs=1, name=f"sums_s{s}"
                    )
                    acc[s] = (out_ps, sums_ps)
                out_ps, sums_ps = acc[s]
                p_sb = p_tile(f"p_{i}")
                src = s_tiles.pop((s, c))
                bias = mb_sb[:, s * NCHUNK + c:s * NCHUNK + c + 1]
                nc.scalar.activation(
                    p_sb,
                    src,
                    mybir.ActivationFunctionType.Exp,
                    bias=bias,
                    scale=INV_SQRT_D,
                )
                if i + 2 < len(stream):
                    s_tiles[stream[i + 2]] = s_mms(*stream[i + 2])
                final_tail = (c == cap - 1) and s == S - 1
                if final_tail and s in run:
                    # Final fragment: fold the pre-final running chain
                    # into the sums PSUM while ACT runs the last exp (PE
                    # is otherwise idle), finish with the last chunk's
                    # probs below — removes the DVE add from the tail
                    # critical path entirely.
                    sums_mms(sums_ps, run.pop(s), True, False)
                vw = vp_w(s, c)
                first, last = c == 0, c == cap - 1
                for h in range(2):
                    nc.tensor.matmul(
                        out_ps[h],
                        vw,
                        p_sb[:, h * 512:(h + 1) * 512],
                        start=first,
                        stop=last,
                    )
                if final_tail:
                    sums_mms(sums_ps, p_sb, cap == 1, True)
                    # h0 cast on Scalar (idle after the last exp) so both
                    # output halves cast concurrently.
                    outn = sb.tile([PART, Q], BF16, tag="outn", bufs=3,
                                   name=f"outn{s}")
                    nc.scalar.copy(outn[:, 0:512], out_ps[0])
                    nc.sync.dma_start(out=out[s][:, 0:512], in_=outn[:, 0:512])
                    nc.vector.tensor_copy(outn[:, 512:1024], out_ps[1])
                    nc.sync.dma_start(
                        out=out[s][:, 512:1024], in_=outn[:, 512:1024]
                    )
                    sums_epilogue(s, sums_ps)
                    continue
                if last:
                    # Epilogue out-copies first: the accumulator bank
                    # frees before the fragment-final DVE add runs. The
                    # out DMAs ride SWDGE: writes on the Sync ring would
                    # queue between the remaining input pieces and starve
                    # the exp stream.
                    outn = sb.tile([PART, Q], BF16, tag="outn", bufs=3,
                                   name=f"outn{s}")
                    nc.vector.tensor_copy(outn[:, 0:512], out_ps[0])
                    nc.gpsimd.dma_start(out=out[s][:, 0:512],
                                        in_=outn[:, 0:512])
                    nc.vector.tensor_copy(outn[:, 512:1024], out_ps[1])
                    nc.gpsimd.dma_start(
                        out=out[s][:, 512:1024], in_=outn[:, 512:1024]
                    )
                # Running-sum chain on DVE: one add per chunk, so only
                # one add remains at the fragment boundary.
                if s not in run:
                    cur = p_sb
                else:
                    cur = p_tile(f"run_{i}")
                    nc.vector.tensor_add(cur, run.pop(s), p_sb)
                if not last:
                    run[s] = cur
                else:
                    # Ship the raw running sum; host reduces partitions.
                    # Sync ring is input-free by the time these fire.
                    nc.sync.dma_start(out=csum[s], in_=cur)

    nc.compile()
    return nc


def _profiles(S, T, maxp):
    """Descending profiles of length S summing to T, parts in [1, maxp]."""
    out = []

    def rec(rem_slots, rem_sum, hi, cur):
        if rem_slots == 0:
            if rem_sum == 0:
                out.append(tuple(cur))
            return
        lo = max(1, rem_sum - (rem_slots - 1) * hi)
        for p in range(min(hi, rem_sum - (rem_slots - 1)), lo - 1, -1):
            cur.append(p)
            rec(rem_slots - 1, rem_sum - p, p, cur)
            cur.pop()

    rec(S, T, maxp, [])
    return out


def _exact_assign(needs, profile):
    """Zero-slack exact cover: cut batches (needs, desc order of (need,
    batch)) into parts exactly matching 8 copies of each profile entry.
    Returns per-batch composition counts over distinct sizes, or None."""
    sizes = sorted(set(profile), reverse=True)
    cap = tuple(8 * profile.count(sz) for sz in sizes)

    comp_cache = {}

    def comps(n):
        if n in comp_cache:
            return comp_cache[n]
        res = []

        def rec(i, rem, cur):
            if rem == 0:
                res.append(tuple(cur) + (0,) * (len(sizes) - len(cur)))
                return
            if i == len(sizes):
                return
            for k in range(rem // sizes[i], -1, -1):
                cur.append(k)
                rec(i + 1, rem - k * sizes[i], cur)
                cur.pop()

        rec(0, n, [])
        comp_cache[n] = res
        return res

    order = sorted(range(len(needs)), key=lambda b: -needs[b])
    fail = set()

    def solve(idx, rem):
        if idx == len(order):
            return [] if all(r == 0 for r in rem) else None
        key = (idx, rem)
        if key in fail:
            return None
        for comp in comps(needs[order[idx]]):
            if all(ci <= ri for ci, ri in zip(comp, rem)):
                tail = solve(idx + 1,
                             tuple(ri - ci for ri, ci in zip(rem, comp)))
                if tail is not None:
                    return [comp] + tail
        fail.add(key)
        return None

    sol = solve(0, cap)
    if sol is None:
        return None
    return sizes, {order[i]: sol[i] for i in range(len(order))}


def _greedy_assign(needs, profile):
    """Slack-tolerant greedy: largest remaining need to largest instance.
    Returns list of (instance_slot_index, batch, frag_len) or None."""
    inst = sorted(
        ((p, s, k) for s, p in enumerate(profile) for k in range(8)),
        reverse=True,
    )
    rem = {b: n for b, n in enumerate(needs)}
    frags = []
    for p, s, k in inst:
        if not rem:
            break
        b = max(rem, key=lambda x: rem[x])
        take = min(p, rem[b])
        frags.append((s, b, take))
        rem[b] -= take
        if rem[b] == 0:
            del rem[b]
    if rem:
        return None
    return frags


def plan(valid_lens: np.ndarray):
    """Choose slot profile + cut batches into fragments.

    Returns (profile_asc, frags) where frags[core][slot] = (b, c0, f)
    (f may be 0 for an empty padded instance).
    """
    need = np.maximum(
        np.minimum((valid_lens.astype(np.int64) + PART - 1) // PART, NCHUNK), 1
    )
    needs = need.tolist()
    total = int(need.sum())
    T0 = -(-total // N_CORES)

    frag_list = None   # list of (slot, batch, frag_len)
    profile = None
    if T0 * N_CORES == total:
        for S in (4, 5, 6):
            # pick the most balanced exact profile (lexicographically
            # smallest in descending representation)
            hits = [(prof, _exact_assign(needs, prof))
                    for prof in _profiles(S, T0, NCHUNK)]
            hits = [(prof, r) for prof, r in hits if r is not None]
            for prof, r in sorted(hits):
                if True:
                    sizes, comp_by_batch = r
                    # expand: fragments per size -> instances
                    by_size = {sz: [] for sz in sizes}
                    for b in sorted(comp_by_batch, key=lambda b: -needs[b]):
                        c0 = 0
                        for sz, cnt in zip(sizes, comp_by_batch[b]):
                            for _ in range(cnt):
                                by_size[sz].append((b, c0, sz))
                                c0 += sz
                    frag_list = []
                    for s, p in enumerate(prof):
                        for (b, c0, f) in by_size[p][:8]:
                            frag_list.append((s, b, c0, f))
                        by_size[p] = by_size[p][8:]
                    profile = prof
                    break
            if frag_list is not None:
                break
    if frag_list is None:
        for T in range(T0, T0 + 4):
            for S in (4, 5, 6):
                done = False
                for prof in _profiles(S, T, NCHUNK):
                    g = _greedy_assign(needs, prof)
                    if g is not None:
                        cursor = {}
                        frag_list = []
                        for (s, b, f) in g:
                            c0 = cursor.get(b, 0)
                            frag_list.append((s, b, c0, f))
                            cursor[b] = c0 + f
                        profile = prof
                        done = True
                        break
                if done:
                    break
            if frag_list is not None:
                break
    # order slots ascending by chunk count (first slot smallest: startup)
    S = len(profile)
    order = sorted(range(S), key=lambda s: profile[s])
    remap = {old: new for new, old in enumerate(order)}
    profile_asc = tuple(profile[s] for s in order)
    per_slot = {s: [] for s in range(S)}
    for (s, b, c0, f) in frag_list:
        per_slot[remap[s]].append((b, c0, f))
    frags = [[None] * S for _ in range(N_CORES)]
    for s in range(S):
        lst = per_slot[s]
        while len(lst) < N_CORES:
            lst.append((0, 0, 0))   # empty padded instance
        for core in range(N_CORES):
            frags[core][s] = lst[core]
    return profile_asc, frags


def host_prep(q, k, v, lens):
    """Shard + lay out inputs for the 8 cores."""
    profile, frags = plan(lens)
    S = len(profile)

    in_maps = []
    for core in range(N_CORES):
        m = {}
        # mask bias table for this core (bf16 is exact enough: bias is
        # 0 or -1e6 -> -999424, and exp underflows to 0 either way)
        mb = np.full((PART, S * NCHUNK), MASK_BIAS, np.float32)
        for s, p in enumerate(profile):
            b, c0, f = frags[core][s]
            L = int(lens[b])
            for j in range(f):
                valid = (c0 + j) * PART + np.arange(PART) < L
                mb[:, s * NCHUNK + j] = np.where(valid, 0.0, MASK_BIAS)
        cst = np.zeros((PART, 4), NP_BF16)
        cst[:, 0] = 1
        cst[:, 3] = 1

        for s, p in enumerate(profile):
            b, c0, f = frags[core][s]
            lo = c0 * PART
            qtT = q[b].T.astype(NP_BF16)
            w = np.zeros((PART, _w_cols(profile, s)), NP_BF16)
            w[:, 0:f * PART] = k[b][lo:lo + f * PART].T
            if s == 0:
                w[:, p * PART:p * PART + Q] = qtT
                off = p * PART + Q
                w[:, off:off + S * NCHUNK] = mb.astype(NP_BF16)
                w[:, off + S * NCHUNK:off + S * NCHUNK + 4] = cst
            else:
                w[:, p * PART:p * PART + 512] = qtT[:, 0:512]
                if 1 <= s <= 3:
                    m[f"qh{s}"] = np.ascontiguousarray(qtT[:, 512:1024])
            m[f"w{s}"] = w
            kv = np.zeros((PART, _kv_cols(profile, s)), NP_BF16)
            voff = 512 if s >= 4 else 0
            if s >= 4:
                kv[:, 0:512] = qtT[:, 512:1024]
            # vp chunk-major: block j = v[(c0+j)*128 : ..., :] (k on part)
            if f:
                kv[:, voff:voff + f * PART] = (
                    v[b][lo:lo + f * PART]
                    .reshape(f, PART, D)
                    .transpose(1, 0, 2)
                    .reshape(PART, f * D)
                )
            m[f"kv{s}"] = kv
        in_maps.append(m)
    return profile, frags, in_maps


def gather(results, profile, frags):
    """Sum partial (out, sums) across fragments; divide + transpose."""
    S = len(profile)
    out_acc = np.zeros((B, PART, Q), np.float32)
    sums_acc = np.zeros((B, Q), np.float32)
    for core in range(N_CORES):
        core_out = np.asarray(results[core]["out"], np.float32)
        core_sums = np.asarray(results[core]["sums"], np.float32)
        core_csum = np.asarray(results[core]["csum"], np.float32)
        for s in range(S):
            b, c0, f = frags[core][s]
            if f == 0:
                continue
            out_acc[b] += core_out[s]
            if s == S - 1:
                sums_acc[b] += core_sums[s].reshape(Q)
            else:
                sums_acc[b] += core_csum[s].sum(axis=0)
    out = np.empty((B, Q, D), np.float32)
    for b in range(B):
        out[b] = (out_acc[b] / sums_acc[b][None, :]).T
    return out


def kernel(queries, keys, values, valid_lens):
    q = np.ascontiguousarray(np.asarray(queries, dtype=np.float32))
    k = np.ascontiguousarray(np.asarray(keys, dtype=np.float32))
    v = np.ascontiguousarray(np.asarray(values, dtype=np.float32))
    lens = np.asarray(valid_lens).astype(np.int64).reshape(B)

    profile, frags, in_maps = host_prep(q, k, v, lens)

    if profile not in _NC_CACHE:
        _NC_CACHE[profile] = build_nc(profile)
    nc = _NC_CACHE[profile]

    res = run_bass_kernel_spmd(nc, in_maps, list(range(N_CORES)))
    return gather(res.results, profile, frags)
